# revision 30
# baseline (speedup 1.0000x reference)
"""Causal self-attention (B=2, S=2048, D=2048, H=16) on 8 TRN2 NeuronCores.

Sharding (data + tensor parallel, per the head-group hint):
  core c -> batch b = c // 4, head group g = c % 4 (heads 4g..4g+3).
  wq/wk/wv are split column-wise per head group (512 cols), wo row-wise
  (512 rows). Each core computes attention for its 4 heads on its batch and
  produces a partial output projection; the host sums the 4 partials per
  batch (the tensor-parallel all-reduce, done at gather time).

All activations stay feature-major ("transposed") so every matmul consumes
operands in their natural layout:
  QT[c,s] = wq.T @ x.T          (lhsT=wq,  rhs=xT)
  KT[c,s] = wk.T @ x.T
  V[s,c]  = x @ wv              (lhsT=xT,  rhs=wv)
  ST[k,q] = K_h Q_h^T           (lhsT=KT_h, rhs=QT_h)
  PT[k,q] = exp(ST*scale - 4); upper triangle of the diagonal 128x128
            sub-block masked by adding a precomputed -30000 triangle
            (one shifted fp16 mask tile) on DVE before the exp
  OT[hd,q]= V_h.T @ PT          (lhsT=V_h, rhs=PT) accumulated in PSUM
  rsum[q] = ones.T @ pacc       where pacc = sum_kb PT_kb is accumulated
            tile-by-tile on DVE right after each exp; the partition
            reduction is then ONE fp16 ones-matmul per (head, qb)
            instead of one matmul per key-block pair (PE off-load)
  out     = (OT/rsum).T @ wo    (lhsT=OT,  rhs=wo)

The three phases are FUSED in issue order: attention for query block qb
is interleaved (block by block) with the projection groups of sequence
block qb+1 (and, for qb=3, with the first 48 output-projection tiles) so
the in-order PE stream always has filler work while the Activation
engine computes exp. The normalized attention output is written into
qt's dead columns (qt[:, h, qb] is only read by qb's own score matmuls),
saving a 2 MB SBUF buffer.

In repeat (timing) mode the next body's input DMA is staggered: each xt
sequence block reloads right after its last reader in the current body
and the weights after the st3 groups, so the 16 MB of input transfers
spread across the whole body instead of contending with the phase-3
output stores; warm bodies skip the st0 partial-pass scheme (inputs are
already resident) and the marginal body cost is the PE-busy time.

Compute dtype fp16 (matmul rel-err ~3e-4); softmax statistics and PSUM
accumulation in fp32; the probs accumulator is fp16 (chain of <=15 adds,
~1e-3 on the denominator, well inside the envelope gate).
"""

import math
from collections import deque

import numpy as np

B = 2
S = 2048
D = 2048
H = 16
HD = 128
N_CORES = 8
NH = 4          # heads per core
C = NH * HD     # 512 per-core projection width
P = 128
DO = D // P     # 16 contraction subtiles
SBLK = 512      # matmul moving free dim / PSUM bank
NSB = S // SBLK  # 4 sequence blocks
NKB = S // P     # 16 key blocks
SCALE = 1.0 / math.sqrt(HD)
EBIAS = -4.0    # constant shift inside exp; cancels in softmax ratio

_STATE = {}


def _build_kernel(repeat=1):
    import concourse.bacc as bacc
    import concourse.mybir as mybir
    import concourse.tile as tile
    from concourse.bass import ts

    F16 = mybir.dt.float16
    F32 = mybir.dt.float32

    nc = bacc.Bacc("TRN2", target_bir_lowering=False, debug=False)

    xt_d = nc.dram_tensor("xt", [D, S], F16, kind="ExternalInput").ap()
    wq_d = nc.dram_tensor("wq", [D, C], F16, kind="ExternalInput").ap()
    wk_d = nc.dram_tensor("wk", [D, C], F16, kind="ExternalInput").ap()
    wv_d = nc.dram_tensor("wv", [D, C], F16, kind="ExternalInput").ap()
    wo_d = nc.dram_tensor("wo", [C, D], F16, kind="ExternalInput").ap()
    out_d = nc.dram_tensor("out", [S, D], F16, kind="ExternalOutput").ap()

    with tile.TileContext(nc) as tc:
        with tc.tile_pool(name="persist", bufs=1) as p_per:
            qt = p_per.tile([P, NH, S], F16)  # also holds normalized out
            kt = p_per.tile([P, NH, S], F16)
            v = p_per.tile([P, DO, C], F16)
            wo_sb = p_per.tile([P, NH, D], F16)
            ones = p_per.tile([P, P], F16)
            ebias = p_per.tile([P, 1], F32)
            masks = p_per.tile([P, SBLK], F16)

            nc.gpsimd.memset(ones[:], 1.0)
            nc.gpsimd.memset(ebias[:], EBIAS)
            # mask for diagonal block a = this mask shifted by 128a cols:
            # sc[:, lo:] += masks[:, :SBLK-lo]
            nc.gpsimd.memset(masks[:], 0.0)
            nc.gpsimd.affine_select(
                out=masks[:],
                in_=masks[:],
                compare_op=mybir.AluOpType.is_ge,
                fill=-30000.0,
                base=0,
                channel_multiplier=-1,
                pattern=[[1, SBLK]],
            )

            with tc.tile_pool(name="xw", bufs=1) as p_xw:
              xt_sb = p_xw.tile([P, DO, S], F16, tag="xt", name="xt")
              wq_sb = p_xw.tile([P, DO, C], F16, tag="wq")
              wk_sb = p_xw.tile([P, DO, C], F16, tag="wk")
              wv_sb = p_xw.tile([P, DO, C], F16, tag="wv")
              xt_rp = xt_d.rearrange("(do p) s -> p do s", p=P)
              wq_r = wq_d.rearrange("(do p) c -> p do c", p=P)
              wk_r = wk_d.rearrange("(do p) c -> p do c", p=P)
              wv_r = wv_d.rearrange("(do p) c -> p do c", p=P)
              ndma = [0]

              def dma(dst, srcap):
                  eng = nc.sync if ndma[0] % 2 == 0 else nc.scalar
                  ndma[0] += 1
                  eng.dma_start(dst, srcap)

              def emit_inputs():
                  # DMA issue order = consumption order; pass 0 at dos-pair
                  # granularity. For rep r+1 this is emitted DURING rep r
                  # (after its last xw reads) so the transfers overlap rep
                  # r's attention/output tail and the next body starts with
                  # all inputs resident.
                  for s2 in (slice(0, 2), slice(2, 4)):
                      dma(wq_sb[:, s2, :], wq_r[:, s2, :])
                      dma(xt_sb[:, s2, 0:SBLK], xt_rp[:, s2, 0:SBLK])
                      dma(wk_sb[:, s2, :], wk_r[:, s2, :])
                      dma(wv_sb[:, s2, :], wv_r[:, s2, :])
                  for pss in range(1, 4):
                      sl = slice(4 * pss, 4 * pss + 4)
                      s01 = slice(4 * pss, 4 * pss + 2)
                      s23 = slice(4 * pss + 2, 4 * pss + 4)
                      dma(wq_sb[:, sl, :], wq_r[:, sl, :])
                      dma(xt_sb[:, s01, 0:SBLK], xt_rp[:, s01, 0:SBLK])
                      dma(wk_sb[:, sl, :], wk_r[:, sl, :])
                      dma(xt_sb[:, s23, 0:SBLK], xt_rp[:, s23, 0:SBLK])
                      dma(wv_sb[:, sl, :], wv_r[:, sl, :])
                  for st in range(1, NSB):
                      dma(xt_sb[:, :, ts(st, SBLK)],
                          xt_rp[:, :, ts(st, SBLK)])
                  nc.sync.dma_start(
                      wo_sb[:], wo_d.rearrange("(cs p) d -> p cs d", p=P))

              emit_inputs()
              # Persistent pools shared by every rep: one PSUM pool holds
              # all three tag rings (sc 3 + av 1 + p1 4 = 8 banks); the
              # out-projection po tiles reuse the p1 ring, so slot reuse
              # across phase and rep boundaries is purely data-dependent
              # (no pool open/close barriers at the rep seam).
              with tc.tile_pool(name="p2w", bufs=7) as p2w, \
                   tc.tile_pool(name="p2acc", bufs=2) as p2acc, \
                   tc.tile_pool(name="p2stat", bufs=2) as p2stat, \
                   tc.tile_pool(name="ps_all", bufs=1, space="PSUM") as ps_all:
               for _rep in range(repeat):

                state = {"ncopy": 0}

                def attention_head(qb, h, feeder):
                    """Emit one head's attention; calls feeder.step() after
                    each key block to interleave filler PE work."""
                    nkb = 4 * (qb + 1)
                    q0 = qb * SBLK
                    av = ps_all.tile([P, SBLK], F32, tag="av", bufs=1, name="av")
                    pend = deque()   # (kb, lo, probs) awaiting AV issue
                    pacc = p2acc.tile([P, SBLK], F16, tag="pacc", name="pacc")

                    def emit_av(kb, lo, probs):
                        nc.tensor.matmul(
                            av[:, lo:],
                            v[:, kb, ts(h, P)],
                            probs[:, lo:],
                            start=(kb == 0), stop=(kb == nkb - 1))

                    for kb in range(nkb):
                        a = kb - 4 * qb
                        lo = P * a if a > 0 else 0
                        sc = ps_all.tile([P, SBLK], F32, tag="sc", bufs=3, name="sc")
                        nc.tensor.matmul(
                            sc[:, lo:],
                            kt[:, h, ts(kb, P)],
                            qt[:, h, q0 + lo:q0 + SBLK],
                            start=True, stop=True)
                        probs = p2w.tile([P, SBLK], F16, tag="probs", name="probs")
                        if a >= 0:
                            # masked scores: -1e9 above the diagonal, added
                            # in place on the PSUM tile before the exp
                            nc.vector.tensor_tensor(
                                sc[:, lo:lo + P], sc[:, lo:lo + P],
                                masks[:, 0:P],
                                op=mybir.AluOpType.add)
                        nc.scalar.activation(
                            probs[:, lo:], sc[:, lo:],
                            mybir.ActivationFunctionType.Exp,
                            bias=ebias[:], scale=SCALE)
                        # running probs accumulator for the softmax
                        # denominator (DVE; kb==0 is always full width)
                        if kb == 0:
                            nc.vector.tensor_copy(pacc[:], probs[:])
                        else:
                            nc.vector.tensor_tensor(
                                pacc[:, lo:], probs[:, lo:], pacc[:, lo:],
                                op=mybir.AluOpType.add)
                        pend.append((kb, lo, probs))
                        if len(pend) > 3:
                            emit_av(*pend.popleft())
                        feeder.step()
                    while pend:
                        emit_av(*pend.popleft())

                    # --- softmax denominator: one partition-reduce matmul ---
                    rs = ps_all.tile([P, SBLK], F32, tag="sc", bufs=3,
                                     name="rs")
                    nc.tensor.matmul(rs[:], ones[:], pacc[:],
                                     start=True, stop=True)
                    rcp = p2stat.tile([P, SBLK], F32, tag="rcp", name="rcp")
                    nc.vector.reciprocal(rcp[:], rs[:])
                    # normalized output aliases into qt's dead columns
                    nc.vector.tensor_tensor(
                        qt[:, h, ts(qb, SBLK)], av[:], rcp[:],
                        op=mybir.AluOpType.mult)

                class Feeder:
                    """Distributes filler thunks across attention blocks."""
                    def __init__(self, thunks, nblocks):
                        self.thunks = deque(thunks)
                        self.per = (len(thunks) / nblocks) if nblocks else 0
                        self.acc = 0.0

                    def step(self):
                        self.acc += self.per
                        while self.acc >= 1.0 and self.thunks:
                            self.thunks.popleft()()
                            self.acc -= 1.0

                    def flush(self):
                        while self.thunks:
                            self.thunks.popleft()()

                # ---------------- phase-1 emitters ----------------
                if True:
                    def group_thunk(kind, i0, i1):
                        def thunk():
                            ps = ps_all.tile([P, SBLK], F32, tag="p1",
                                             bufs=4, name="p1ps")
                            for do in range(DO):
                                fl = do == 0
                                ll = do == DO - 1
                                if kind == "q":
                                    nc.tensor.matmul(
                                        ps[:], wq_sb[:, do, ts(i0, P)],
                                        xt_sb[:, do, ts(i1, SBLK)],
                                        start=fl, stop=ll)
                                elif kind == "k":
                                    nc.tensor.matmul(
                                        ps[:], wk_sb[:, do, ts(i0, P)],
                                        xt_sb[:, do, ts(i1, SBLK)],
                                        start=fl, stop=ll)
                                else:
                                    nc.tensor.matmul(
                                        ps[:], xt_sb[:, do, ts(i0, P)],
                                        wv_sb[:, do, :],
                                        start=fl, stop=ll)
                            if kind == "q":
                                dst = qt[:, i0, ts(i1, SBLK)]
                            elif kind == "k":
                                dst = kt[:, i0, ts(i1, SBLK)]
                            else:
                                dst = v[:, i0, :]
                            # alternate ACT/DVE for the PSUM->SBUF eviction
                            if state["ncopy"] % 2 == 0:
                                nc.scalar.copy(dst, ps[:])
                            else:
                                nc.vector.tensor_copy(dst, ps[:])
                            state["ncopy"] += 1
                        return thunk

                    def st_groups(st):
                        gs = []
                        for ct in range(NH):
                            gs.append(group_thunk("q", ct, st))
                        for ct in range(NH):
                            gs.append(group_thunk("k", ct, st))
                        for sv in range(4 * st, 4 * st + 4):
                            gs.append(group_thunk("v", sv, 0))
                        return gs

                    # st=0 projections of the FIRST rep run while the input
                    # DMA is still streaming in: split the contraction into
                    # 4 passes of 4 d-subtiles each (with fp16 SBUF
                    # partials) so PE only ever waits for the next 4 xt
                    # tiles, not all 16. Later reps have all inputs
                    # prefetched and use plain groups.
                    st0 = [("q", ct, 0) for ct in range(NH)] + \
                          [("k", ct, 0) for ct in range(NH)] + \
                          [("v", sv, 0) for sv in range(4)]
                    if _rep > 0:
                        for kind, i0, i1 in st0:
                            group_thunk(kind, i0, i1)()
                    else:
                      with tc.tile_pool(name="p1part", bufs=12) as p1part:
                        parts = {}
                        for pss in range(4):
                            dos = range(4 * pss, 4 * pss + 4)
                            for c0 in range(0, 12, 2):
                                chunk = list(enumerate(st0))[c0:c0 + 2]
                                pss_ps = {}
                                for gi, _ in chunk:
                                    pss_ps[gi] = ps_all.tile(
                                        [P, SBLK], F32, tag="p1", bufs=4,
                                        name="ps")
                                # do-major so the in-order PE stream consumes
                                # each xt subtile as soon as its DMA lands
                                for do in dos:
                                    fl = do == 4 * pss
                                    ll = do == 4 * pss + 3
                                    for gi, (kind, i0, i1) in chunk:
                                        ps = pss_ps[gi]
                                        if kind == "q":
                                            nc.tensor.matmul(
                                                ps[:],
                                                wq_sb[:, do, ts(i0, P)],
                                                xt_sb[:, do, ts(i1, SBLK)],
                                                start=fl, stop=ll)
                                        elif kind == "k":
                                            nc.tensor.matmul(
                                                ps[:],
                                                wk_sb[:, do, ts(i0, P)],
                                                xt_sb[:, do, ts(i1, SBLK)],
                                                start=fl, stop=ll)
                                        else:
                                            nc.tensor.matmul(
                                                ps[:],
                                                xt_sb[:, do, ts(i0, P)],
                                                wv_sb[:, do, :],
                                                start=fl, stop=ll)
                                for gi, (kind, i0, i1) in chunk:
                                    ps = pss_ps[gi]
                                    # st0 evictions stay on DVE: the ACT
                                    # sequencer is serialized behind the
                                    # input dma_start dispatches early on
                                    if pss == 0:
                                        pt = p1part.tile([P, SBLK], F16,
                                                         tag="pt", name="pt")
                                        parts[gi] = pt
                                        nc.vector.tensor_copy(pt[:], ps[:])
                                    else:
                                        if kind == "q":
                                            dst = qt[:, i0, ts(i1, SBLK)]
                                        elif kind == "k":
                                            dst = kt[:, i0, ts(i1, SBLK)]
                                        else:
                                            dst = v[:, i0, :]
                                        nc.vector.tensor_tensor(
                                            (parts[gi][:] if pss < 3 else dst),
                                            ps[:], parts[gi][:],
                                            op=mybir.AluOpType.add)
                    # Next-rep input prefetch is STAGGERED: each xt block
                    # reloads right after its last reader in this rep, so
                    # the 16 MB of input DMA spreads across the whole body
                    # instead of contending with the phase-3 output stores
                    # in the last quarter.
                    if _rep < repeat - 1:
                        dma(xt_sb[:, :, 0:SBLK], xt_rp[:, :, 0:SBLK])
                    # attention qb=s interleaves with st=s+1 projections
                    for s in range(NSB - 1):
                        feeder = Feeder(st_groups(s + 1), 16 * (s + 1))
                        for h in range(NH):
                            attention_head(s, h, feeder)
                        feeder.flush()
                        if _rep < repeat - 1:
                            dma(xt_sb[:, :, ts(s + 1, SBLK)],
                                xt_rp[:, :, ts(s + 1, SBLK)])

                    if _rep < repeat - 1:
                        dma(wq_sb[:], wq_r[:])
                        dma(wk_sb[:], wk_r[:])
                        dma(wv_sb[:], wv_r[:])
                        nc.sync.dma_start(
                            wo_sb[:],
                            wo_d.rearrange("(cs p) d -> p cs d", p=P))

                # ---------------- phase 3 + last attention block ----------
                with tc.tile_pool(name="p3stage", bufs=3) as p3stage:
                    stage_cur = {}

                    def p3_tile(so, no):
                        def thunk():
                            # po shares the p1 PSUM ring (phase 1 is done
                            # with it by now; reuse is data-dependent)
                            po = ps_all.tile([P, SBLK], F32, tag="p1",
                                             bufs=4, name="po")
                            for cs in range(NH):
                                nc.tensor.matmul(
                                    po[:],
                                    qt[:, cs, ts(so, P)],
                                    wo_sb[:, cs, ts(no, SBLK)],
                                    start=(cs == 0), stop=(cs == NH - 1))
                            if no == 0:
                                stage_cur[0] = p3stage.tile(
                                    [P, NSB, SBLK], F16, tag="st", name="st")
                            stage = stage_cur[0]
                            if (so * NSB + no) % 2 == 0:
                                nc.scalar.copy(stage[:, no, :], po[:])
                            else:
                                nc.vector.tensor_copy(stage[:, no, :], po[:])
                            if so < 12:
                                if no == NSB - 1:
                                    # one wide row-store per so keeps the SP
                                    # queue sequencer (~1.3 us per dma_start)
                                    # off the critical path
                                    nc.sync.dma_start(
                                        out_d[ts(so, P), :], stage[:])
                            elif no % 2 == 1:
                                # tail rows: half-row stores fanned across
                                # both queues so the final drain overlaps
                                # the last matmuls (ACT's sequencer is
                                # exp-free by now)
                                eng = nc.sync if no == 1 else nc.scalar
                                eng.dma_start(
                                    out_d[ts(so, P), (no - 1) * SBLK:
                                          (no + 1) * SBLK],
                                    stage[:, no - 1:no + 1, :])
                        return thunk

                    early = [p3_tile(so, no)
                             for so in range(12) for no in range(NSB)]
                    feeder = Feeder(early, 16 * NSB)
                    for h in range(NH):
                        attention_head(NSB - 1, h, feeder)
                    feeder.flush()
                    for so in range(12, NKB):
                        for no in range(NSB):
                            p3_tile(so, no)()

    nc.compile()
    return nc


def _shard_inputs(x, wq, wk, wv, wo):
    in_maps = []
    for c in range(N_CORES):
        b, g = divmod(c, NH)
        cols = slice(g * C, (g + 1) * C)
        in_maps.append({
            "xt": np.ascontiguousarray(x[b].T).astype(np.float16),
            "wq": wq[:, cols].astype(np.float16),
            "wk": wk[:, cols].astype(np.float16),
            "wv": wv[:, cols].astype(np.float16),
            "wo": np.ascontiguousarray(wo[cols, :]).astype(np.float16),
        })
    return in_maps


def kernel(x, wq, wk, wv, wo):
    from concourse.bass_utils import run_bass_kernel_spmd

    if "nc" not in _STATE:
        _STATE["nc"] = _build_kernel()
    nc = _STATE["nc"]

    in_maps = _shard_inputs(
        np.asarray(x), np.asarray(wq), np.asarray(wk),
        np.asarray(wv), np.asarray(wo))
    res = run_bass_kernel_spmd(nc, in_maps, core_ids=list(range(N_CORES)))
    out = np.zeros((B, S, D), dtype=np.float32)
    for c in range(N_CORES):
        b = c // NH
        out[b] += res.results[c]["out"]
    return out

